# revision 36
# baseline (speedup 1.0000x reference)
"""Trainium2 Bass kernel for nn_PizzaBurningEffect.

Reference (per batch b):
    ew[h,w]   : fixed edge-weight grid (input-independent)
    spots     = max_s  gy_s[h] * gx_s[w]          (separable Gaussians)
    m         = max(ew, spots)
    out[c]    = img[c] * (1 + fsc_{b,c} * m),     fsc = -burn_b*(1-dark_c)
(The reference clips are no-ops: every operand is in [0,1) and bm <= 0.8.)

Device algorithm (p-norm max):
    max_s z_s  ==  (sum_s z_s^p)^(1/p)   when no two z_s tie at a large value.
    The host partitions the 8 spots into G=3 groups so that no intra-group
    pair ever ties above ~0.1 (validated per input), then the TensorEngine
    computes the per-group sums  plane_g = sum_{s in g} gy_s^p * gx_s^p
    as K=8 matmuls of the ^p 1-D tables (p=16).  DVE merges the G planes
    and ew^p with maxes *in the power domain* (max is monotone under
    x^(1/p)), ACT computes m = exp(ln(.)/p) once per chunk, DVE computes
    f_c = 1 + fsc_c*m and the blend out = img * f in bf16.

    img/out are cast fp32<->bf16 during the DMA itself (SWDGE), so HBM
    traffic is the fp32 tensors but on-chip everything is 16-bit.

Error budget (validated on the reference inputs end to end): bf16 img/out
quantization ~8e-3, grouping deviation < 2e-3, ln/exp path < 1e-3;
total < 1.2e-2 vs the 2e-2 gate.  fp16 is NOT usable for img/out: fp16
subnormal spacing (6e-8) vs the metric's 1e-6 denominator floor gives
2e-2 error on its own.

Sharding: pure data parallel, 4 batches per core on 8 cores.
"""

import numpy as np
import ml_dtypes

import concourse.bacc as bacc
import concourse.bass as bass
from concourse import mybir
from concourse.tile import TileContext
from concourse.bass_utils import run_bass_kernel_spmd

# Make the act-table pass resolve both Ln and Exp to the one set that
# contains them both ("natural_log_exp_and_others"); the default order
# resolves them to two different sets, and the per-chunk Ln/Exp
# alternation then reloads tables every chunk (~1.5us each).
_orig_get_act_tables = bacc.get_activation_tables


def _pinned_act_tables(arch):
    # The entry ORDER must stay aligned with act_info.json (the index is the
    # act_func_set_id walrus resolves), so instead of reordering, strip
    # Ln/Exp from every other set so only the combined set can be chosen.
    tabs = _orig_get_act_tables(arch)
    key = "natural_log_exp_and_others"
    if key not in tabs:
        return tabs
    ln_exp = {mybir.ActivationFunctionType.Ln,
              mybir.ActivationFunctionType.Exp}
    return {name: (fns if name == key else fns - ln_exp)
            for name, fns in tabs.items()}


bacc.get_activation_tables = _pinned_act_tables

B, C, H, W, S = 32, 3, 512, 512, 8
NCORES = 8
BL = B // NCORES          # batches per core
P = 128                   # partitions
K = H // P                # row chunks per image
F32 = mybir.dt.float32
BF16 = mybir.dt.bfloat16
FP16 = mybir.dt.float16

BURN_MIN, BURN_MAX = 0.2, 0.8
DARK = np.array([0.7, 0.4, 0.3], dtype=np.float64)

PNORM = 16                # the p in the p-norm max
# The ACT Ln spline mishandles inputs below ~2^-64 (clamps, giving phantom
# mask ~0.06), so the power-domain values are pre-scaled by 2^SCALE_K and
# floored: m16*2^K stays in [2^-59, 2^62].  The scale is divided back out
# via Exp's free bias.  The floor makes the mask saturate at
# (2^-118)^(1/16) = 0.006 from below, well inside the error budget.
SCALE_K = 59
EW_FLOOR = 2.0 ** (-118)
# Validation accepts a grouping when the per-pixel OUTPUT error bound
# 0.7*burn*|m_apx-m_true| / (1 - 0.7*burn*m_true) stays under OUT_BUDGET;
# together with the ~0.8e-2 bf16 img/out quantization floor that keeps the
# end-to-end rel err under the 2e-2 gate.  Inputs that need more get a
# bigger G (recompile).
OUT_BUDGET = 1.15e-2


def _build_program(G):
    nc = bacc.Bacc("TRN2", target_bir_lowering=False, debug=False,
                   num_devices=NCORES)

    img = nc.dram_tensor("img", [BL, C, H, W], F32, kind="ExternalInput")
    gyT = nc.dram_tensor("gyT", [S, BL, K, P], BF16, kind="ExternalInput")
    gxg = nc.dram_tensor("gxg", [S, BL, G * W], BF16, kind="ExternalInput")
    ew16 = nc.dram_tensor("ew16", [P, K, W], BF16, kind="ExternalInput")
    fsc = nc.dram_tensor("fsc", [P, BL, C], F32, kind="ExternalInput")
    out = nc.dram_tensor("out", [BL, C, H, W], F32, kind="ExternalOutput")

    img_r = img.rearrange("b c (k p) w -> b p k c w", p=P)
    out_r = out.rearrange("b c (k p) w -> b p k c w", p=P)

    mx = mybir.AluOpType.max
    mult = mybir.AluOpType.mult
    add = mybir.AluOpType.add

    with TileContext(nc) as tc:
        with (
            tc.tile_pool(name="singles", bufs=1) as singles,
            tc.tile_pool(name="imgp", bufs=4) as imgp,
            tc.tile_pool(name="outp", bufs=6) as outp,
            tc.tile_pool(name="m16p", bufs=3) as m16p,
            tc.tile_pool(name="lnp", bufs=3) as lnp,
            tc.tile_pool(name="mbp", bufs=3) as mbp,
            tc.tile_pool(name="fp", bufs=4) as fp,
            tc.tile_pool(name="pp", bufs=2, space="PSUM") as pp,
        ):
            gyT_t = singles.tile([S, BL, K, P], BF16)
            nc.sync.dma_start(out=gyT_t[:], in_=gyT[:])
            gxg_t = singles.tile([S, BL, G * W], BF16)
            nc.sync.dma_start(out=gxg_t[:], in_=gxg[:])
            ew_t = singles.tile([P, K, W], BF16)
            nc.sync.dma_start(out=ew_t[:, 0, :], in_=ew16[:, 0, :])
            fsc_t = singles.tile([P, BL, C], F32)
            nc.sync.dma_start(out=fsc_t[:], in_=fsc[:])
            for k in range(1, K):
                nc.sync.dma_start(out=ew_t[:, k, :], in_=ew16[:, k, :])
            # Warm the ACT table (ln/exp set) so its ~2.7us load overlaps
            # the initial DMAs instead of stalling the first real Ln.
            warm = singles.tile([P, 1], FP16)
            nc.scalar.activation(out=warm[:], in_=ew_t[:, 0, 0:1],
                                 func=mybir.ActivationFunctionType.Ln,
                                 bias=0.0, scale=1.0)

            # the 2^SCALE_K range shift is divided back out via fsc (host
            # folds 2^(-K/p) into the fsc scalars), so Exp needs no bias.
            # Ln/Exp run per half-batch (2 chunks) -- big enough to amortize
            # the ACT fixed cost, small enough to keep the pipeline draining.
            KH = 4
            PD = 2   # img prefetch distance (batches)

            # img loads are issued PD batches ahead of their compute so the
            # single SWDGE queue never stalls them behind out-stores (the Q7
            # sequencer waits each DMA's semaphore IN ORDER, so an out-store
            # waiting on its blend would otherwise starve later img loads).
            imgb_tiles = {}

            def issue_img(bb):
                t = imgp.tile([P, K, C * W], BF16, name="imgb")
                for k in range(K):
                    nc.gpsimd.dma_start(out=t[:, k, :],
                                        in_=img_r[bb, :, k])  # f32->bf16
                imgb_tiles[bb] = t

            for bb in range(min(PD, BL)):
                issue_img(bb)

            m_tiles = {}

            def merge_stage(b):
                """matmuls + power-domain merges + Ln/Exp for batch b."""
                m16b = m16p.tile([P, K, W], F32, name="m16b")
                for k in range(K):
                    ps = pp.tile([P, G, W], F32, name="ps")
                    for g in range(G):
                        nc.tensor.matmul(
                            out=ps[:, g, :], lhsT=gyT_t[:, b, k, :],
                            rhs=gxg_t[:, b, g * W:(g + 1) * W],
                            start=True, stop=True)
                    # serial max chain, one PSUM operand per op (PSUM has a
                    # single DVE read port)
                    if G > 1:
                        t = m16p.tile([P, G - 1, W], F32, tag="mrg",
                                      name="mrg")
                    prev = ew_t[:, k, :]
                    for g in range(G):
                        dst = m16b[:, k, :] if g == G - 1 else t[:, g, :]
                        nc.vector.tensor_tensor(
                            out=dst, in0=ps[:, g, :], in1=prev, op=mx)
                        prev = dst
                # m = exp(ln(.)/p); fp16 keeps |ln| <= 44 precise
                lnv = lnp.tile([P, K * W], FP16, name="lnv")
                nc.scalar.activation(
                    out=lnv[:], in_=m16b[:].rearrange("p k w -> p (k w)"),
                    func=mybir.ActivationFunctionType.Ln,
                    bias=0.0, scale=1.0)
                m_t = mbp.tile([P, K, W], BF16, name="m_t")
                nc.scalar.activation(
                    out=m_t[:].rearrange("p k w -> p (k w)"), in_=lnv[:],
                    func=mybir.ActivationFunctionType.Exp,
                    bias=0.0, scale=1.0 / PNORM)
                m_tiles[b] = m_t

            def blend_stage(b):
                """f + blend + out-store for batch b (runs one batch behind
                merge_stage so DVE never waits on ACT)."""
                imgb = imgb_tiles.pop(b)
                m_t = m_tiles.pop(b)
                for k in range(K):
                    # f_c = fsc_c * m + 1   (tensor_scalar)
                    f_t = fp.tile([P, C, W], BF16, name="f_t")
                    for c in range(C):
                        nc.vector.tensor_scalar(
                            out=f_t[:, c, :], in0=m_t[:, k, :],
                            scalar1=fsc_t[:, b, c:c + 1], scalar2=1.0,
                            op0=mult, op1=add)
                    # out = img * f  (one wide bf16 tt, 2x packed)
                    outb = outp.tile([P, C * W], BF16, name="outb")
                    nc.vector.tensor_tensor(
                        out=outb[:], in0=imgb[:, k, :],
                        in1=f_t[:].rearrange("p c w -> p (c w)"), op=mult)
                    nc.gpsimd.dma_start(out=out_r[b, :, k],
                                        in_=outb[:])  # bf16->f32

            for b in range(BL):
                if b + PD < BL:
                    issue_img(b + PD)
                merge_stage(b)
                if b >= 1:
                    blend_stage(b - 1)
            blend_stage(BL - 1)

    nc.compile()
    return nc


_NC_CACHE = {}


def _get_nc(G):
    if G not in _NC_CACHE:
        _NC_CACHE[G] = _build_program(G)
    return _NC_CACHE[G]


def _edge_weight16():
    y = np.linspace(-1.0, 1.0, H)
    x = np.linspace(-1.0, 1.0, W)
    yc, xc = np.meshgrid(y, x, indexing="ij")
    dist = np.sqrt(xc ** 2 + yc ** 2)
    ew = np.exp(2.0 * (dist - 0.7))
    ew = (ew - ew.min()) / (ew.max() - ew.min() + 1e-6)
    ew16 = np.maximum(ew ** PNORM, EW_FLOOR) * 2.0 ** SCALE_K
    lay = ew16.reshape(K, P, W).transpose(1, 0, 2).astype(ml_dtypes.bfloat16)
    return ew, np.ascontiguousarray(lay)


_EW = None          # (ew float64 [H,W], ew16 device layout [P,K,W] f32)


def _spot_tables(u_xy, u_radius, u_intensity):
    """gy[b,s,h], gx[b,s,w] (float64, sint folded into gx) and their ^p."""
    u_xy = np.asarray(u_xy, np.float64)
    u_radius = np.asarray(u_radius, np.float64)
    u_intensity = np.asarray(u_intensity, np.float64)

    y = np.linspace(-1.0, 1.0, H)
    x = np.linspace(-1.0, 1.0, W)
    spot_xy = 2.0 * u_xy - 1.0
    sx = spot_xy[..., 0]
    sy = spot_xy[..., 1]
    radius = 0.05 + 0.15 * u_radius
    sint = 0.5 + 0.5 * u_intensity
    inv2r2 = 1.0 / (2.0 * radius ** 2)

    dy2 = (y[None, None, :] - sy[..., None]) ** 2 * inv2r2[..., None]
    dx2 = (x[None, None, :] - sx[..., None]) ** 2 * inv2r2[..., None]
    gy = np.exp(-dy2)
    gx = np.exp(-dx2) * sint[..., None]
    # ^p computed in the exponent (no underflow-then-power double rounding)
    gyp = np.exp(-PNORM * dy2)
    gxp = np.exp(np.clip(-PNORM * dx2 + PNORM * np.log(sint[..., None]),
                         -745.0, 0.0))
    return gy, gx, gyp, gxp


def _out_err(m_apx, m_true, burn_b):
    fb = 0.7 * burn_b
    return np.max(np.abs(fb * (m_apx - m_true))
                  / (1.0 - fb * np.minimum(m_true, 1.0)))


def _group_spots(gyp_b, gxp_b, gy_b, gx_b, ew, G, burn_b):
    """Partition the 8 spots into <=G groups so the grouped p-norm tracks the
    true max.  Greedy coloring on a coarse conflict graph, then exhaustive
    search (minimizing the output-error bound) if greedy needs more than G
    colors."""
    ss = slice(0, H, 4)
    spots = gy_b[:, ss, None] * gx_b[:, None, ss]        # [S, 128, 128]
    sp = gyp_b[:, ss, None] * gxp_b[:, None, ss]         # == spots^p

    for thr in (0.006, 0.010):
        conflict = np.zeros((S, S), bool)
        for i in range(S):
            for j in range(i + 1, S):
                dev = (sp[i] + sp[j]) ** (1.0 / PNORM) \
                    - np.maximum(spots[i], spots[j])
                conflict[i, j] = conflict[j, i] = dev.max() > thr
        order = np.argsort(-conflict.sum(1))
        colors = -np.ones(S, int)
        ok = True
        for s in order:
            used = {colors[t] for t in range(S)
                    if conflict[s, t] and colors[t] >= 0}
            c0 = 0
            while c0 in used:
                c0 += 1
            if c0 >= G:
                ok = False
                break
            colors[s] = c0
        if ok:
            return colors

    # exhaustive over G-colorings (canonical: spot 0 in group 0), vectorized
    s8 = slice(0, 128, 2)          # stride-8 grid overall
    spc = spots[:, s8, :][:, :, s8].reshape(S, -1)
    spp = sp[:, s8, :][:, :, s8].reshape(S, -1)
    tmax = spc.max(0)
    best_dev, best_colors = np.inf, None
    n_assign = G ** (S - 1)
    for code in range(n_assign):
        colors = [0]
        c = code
        for _ in range(S - 1):
            colors.append(c % G)
            c //= G
        colors = np.array(colors)
        planes = np.stack([
            spp[colors == g].sum(0) if (colors == g).any()
            else np.zeros_like(tmax)
            for g in range(G)])
        dev = _out_err(planes.max(0) ** (1.0 / PNORM), tmax, burn_b)
        if dev < best_dev:
            best_dev, best_colors = dev, colors
    return best_colors


def _validate(colors, gyp_b, gxp_b, gy_b, gx_b, ew, G, burn_b):
    """stride-2 full check: grouped p-norm + ew vs true max + ew, scored as
    a bound on the output rel-err contribution."""
    ss = slice(0, H, 2)
    ew_s = ew[ss, :][:, ss]
    m_true = (gy_b[:, ss, None] * gx_b[:, None, ss]).max(0)
    m_true = np.maximum(m_true, ew_s)
    planes = np.stack([
        np.einsum('sh,sw->hw', gyp_b[colors == g][:, ss],
                  gxp_b[colors == g][:, ss])
        if (colors == g).any() else np.zeros_like(ew_s)
        for g in range(G)])
    m16 = np.maximum(planes.max(0), np.maximum(ew_s ** PNORM, EW_FLOOR))
    m_apx = m16 ** (1.0 / PNORM)
    return _out_err(m_apx, m_true, burn_b) <= OUT_BUDGET


def _host_tables(u_xy, u_radius, u_intensity, u_burn, ew):
    gy, gx, gyp, gxp = _spot_tables(u_xy, u_radius, u_intensity)
    u_burn = np.asarray(u_burn, np.float64)
    burn = BURN_MIN + (BURN_MAX - BURN_MIN) * u_burn
    fsc = -(burn[:, None] * (1.0 - DARK)[None, :])       # [B,C]

    G = 2
    while True:
        all_colors = np.zeros((B, S), int)
        ok = True
        for b in range(B):
            colors = _group_spots(gyp[b], gxp[b], gy[b], gx[b], ew, G,
                                  burn[b])
            if colors is None or not _validate(
                    colors, gyp[b], gxp[b], gy[b], gx[b], ew, G, burn[b]):
                ok = False
                break
            all_colors[b] = colors
        if ok:
            break
        G += 1                      # correctness escape hatch; recompiles
        if G > S:
            raise RuntimeError("spot grouping failed")

    # device layouts
    gyT_lay = np.ascontiguousarray(
        gyp.reshape(B, S, K, P).transpose(1, 0, 2, 3)
    ).astype(ml_dtypes.bfloat16)                          # [S,B,K,P]
    gxg_lay = np.zeros((S, B, G * W), dtype=ml_dtypes.bfloat16)
    gxp_dev = gxp * 2.0 ** SCALE_K      # range shift for the device Ln
    for b in range(B):
        for s in range(S):
            g = all_colors[b, s]
            gxg_lay[s, b, g * W:(g + 1) * W] = gxp_dev[b, s].astype(
                ml_dtypes.bfloat16)
    # fold the 2^(-SCALE_K/p) un-shift of the device mask into fsc
    fsc = fsc * 2.0 ** (-SCALE_K / PNORM)
    fsc_lay = np.broadcast_to(fsc.astype(np.float32), (P, B, C))
    return G, gyT_lay, gxg_lay, np.ascontiguousarray(fsc_lay)


def kernel(img, u_xy, u_radius, u_intensity, u_burn, _run_kwargs=None):
    global _EW
    img = np.ascontiguousarray(np.asarray(img, np.float32))
    if _EW is None:
        _EW = _edge_weight16()
    ew, ew16_lay = _EW

    G, gyT_lay, gxg_lay, fsc_lay = _host_tables(
        u_xy, u_radius, u_intensity, u_burn, ew)

    nc = _get_nc(G)
    core_ids = list(range(NCORES))
    in_maps = []
    for i in core_ids:
        lo, hi = i * BL, (i + 1) * BL
        in_maps.append({
            "img": img[lo:hi],
            "gyT": np.ascontiguousarray(gyT_lay[:, lo:hi]),
            "gxg": np.ascontiguousarray(gxg_lay[:, lo:hi]),
            "ew16": ew16_lay,
            "fsc": np.ascontiguousarray(fsc_lay[:, lo:hi]),
        })
    res = run_bass_kernel_spmd(nc, in_maps, core_ids, **(_run_kwargs or {}))
    out = np.concatenate([res.results[i]["out"] for i in core_ids], axis=0)
    if _run_kwargs:
        kernel._last_results = res
    return out


# revision 37
# speedup vs baseline: 1.1671x; 1.1671x over previous
"""Trainium2 Bass kernel for nn_PizzaBurningEffect.

Reference (per batch b):
    ew[h,w]   : fixed edge-weight grid (input-independent)
    spots     = max_s  gy_s[h] * gx_s[w]          (separable Gaussians)
    m         = max(ew, spots)
    out[c]    = img[c] * (1 + fsc_{b,c} * m),     fsc = -burn_b*(1-dark_c)
(The reference clips are no-ops: every operand is in [0,1) and bm <= 0.8.)

Device algorithm (p-norm max):
    max_s z_s  ==  (sum_s z_s^p)^(1/p)   when no two z_s tie at a large value.
    The host partitions the 8 spots into G groups (G=2 for the reference
    inputs; escalates per input until a per-pixel output-error bound is
    met), then the TensorEngine computes the per-group sums
    plane_g = sum_{s in g} gy_s^p * gx_s^p  as K=8 matmuls of the ^p 1-D
    tables (p=16).  DVE merges the G planes and ew^p with maxes *in the
    power domain* (max is monotone under x^(1/p)), ACT computes
    m = exp(ln(.)/p) once per batch, DVE computes f_c = 1 + fsc_c*m and
    the blend out = img * f in bf16.

    img/out are cast fp32<->bf16 during the DMA itself (SWDGE), so HBM
    traffic is the fp32 tensors but on-chip everything is 16-bit.  The
    kernel is HBM-bound: 25.8 MB/core at ~358 GB/s ~= 72 us is the floor;
    a 2-stage software pipeline (blend lags merge by one batch) plus
    2-batch img prefetch keep the single SWDGE queue from head-of-line
    blocking.

Error budget (measured on the reference inputs end to end): bf16 img/out
quantization ~8e-3, p-norm grouping deviation <= 1.1e-2 bound per pixel;
measured total 1.53e-2 vs the 2e-2 gate.  fp16 is NOT usable for img/out:
fp16 subnormal spacing (6e-8) vs the metric's 1e-6 denominator floor
gives 2e-2 error on its own.

Sharding: pure data parallel, 4 batches per core on 8 cores.
"""

import numpy as np
import ml_dtypes

import concourse.bacc as bacc
import concourse.bass as bass
from concourse import mybir
from concourse.tile import TileContext
from concourse.bass_utils import run_bass_kernel_spmd

# Make the act-table pass resolve both Ln and Exp to the one set that
# contains them both ("natural_log_exp_and_others"); the default order
# resolves them to two different sets, and the per-chunk Ln/Exp
# alternation then reloads tables every chunk (~1.5us each).
_orig_get_act_tables = bacc.get_activation_tables


def _pinned_act_tables(arch):
    # The entry ORDER must stay aligned with act_info.json (the index is the
    # act_func_set_id walrus resolves), so instead of reordering, strip
    # Ln/Exp from every other set so only the combined set can be chosen.
    tabs = _orig_get_act_tables(arch)
    key = "natural_log_exp_and_others"
    if key not in tabs:
        return tabs
    ln_exp = {mybir.ActivationFunctionType.Ln,
              mybir.ActivationFunctionType.Exp}
    return {name: (fns if name == key else fns - ln_exp)
            for name, fns in tabs.items()}


bacc.get_activation_tables = _pinned_act_tables

B, C, H, W, S = 32, 3, 512, 512, 8
NCORES = 8
BL = B // NCORES          # batches per core
P = 128                   # partitions
K = H // P                # row chunks per image
F32 = mybir.dt.float32
BF16 = mybir.dt.bfloat16
FP16 = mybir.dt.float16

BURN_MIN, BURN_MAX = 0.2, 0.8
DARK = np.array([0.7, 0.4, 0.3], dtype=np.float64)

PNORM = 16                # the p in the p-norm max
# The ACT Ln spline mishandles inputs below ~2^-64 (clamps, giving phantom
# mask ~0.06), so the power-domain values are pre-scaled by 2^SCALE_K and
# floored: m16*2^K stays in [2^-59, 2^62].  The scale is divided back out
# via Exp's free bias.  The floor makes the mask saturate at
# (2^-118)^(1/16) = 0.006 from below, well inside the error budget.
SCALE_K = 59
EW_FLOOR = 2.0 ** (-118)
# Validation accepts a grouping when the per-pixel OUTPUT error bound
# 0.7*burn*|m_apx-m_true| / (1 - 0.7*burn*m_true) stays under OUT_BUDGET;
# together with the ~0.8e-2 bf16 img/out quantization floor that keeps the
# end-to-end rel err under the 2e-2 gate.  Inputs that need more get a
# bigger G (recompile).
OUT_BUDGET = 1.15e-2


def _build_program(G):
    nc = bacc.Bacc("TRN2", target_bir_lowering=False, debug=False,
                   num_devices=NCORES)

    img = nc.dram_tensor("img", [BL, C, H, W], F32, kind="ExternalInput")
    gyT = nc.dram_tensor("gyT", [S, BL, K, P], BF16, kind="ExternalInput")
    gxg = nc.dram_tensor("gxg", [S, BL, G * W], BF16, kind="ExternalInput")
    ew16 = nc.dram_tensor("ew16", [P, K, W], BF16, kind="ExternalInput")
    fsc = nc.dram_tensor("fsc", [P, BL, C], F32, kind="ExternalInput")
    out = nc.dram_tensor("out", [BL, C, H, W], F32, kind="ExternalOutput")

    img_r = img.rearrange("b c (k p) w -> b p k c w", p=P)
    out_r = out.rearrange("b c (k p) w -> b p k c w", p=P)

    mx = mybir.AluOpType.max
    mult = mybir.AluOpType.mult
    add = mybir.AluOpType.add

    with TileContext(nc) as tc:
        with (
            tc.tile_pool(name="singles", bufs=1) as singles,
            tc.tile_pool(name="imgp", bufs=4) as imgp,
            tc.tile_pool(name="outp", bufs=6) as outp,
            tc.tile_pool(name="m16p", bufs=3) as m16p,
            tc.tile_pool(name="lnp", bufs=3) as lnp,
            tc.tile_pool(name="mbp", bufs=3) as mbp,
            tc.tile_pool(name="fp", bufs=4) as fp,
            tc.tile_pool(name="pp", bufs=2, space="PSUM") as pp,
        ):
            gyT_t = singles.tile([S, BL, K, P], BF16)
            nc.sync.dma_start(out=gyT_t[:], in_=gyT[:])
            gxg_t = singles.tile([S, BL, G * W], BF16)
            nc.sync.dma_start(out=gxg_t[:], in_=gxg[:])
            ew_t = singles.tile([P, K, W], BF16)
            nc.sync.dma_start(out=ew_t[:, 0, :], in_=ew16[:, 0, :])
            fsc_t = singles.tile([P, BL, C], F32)
            nc.sync.dma_start(out=fsc_t[:], in_=fsc[:])
            for k in range(1, K):
                nc.sync.dma_start(out=ew_t[:, k, :], in_=ew16[:, k, :])
            # Warm the ACT table (ln/exp set) so its ~2.7us load overlaps
            # the initial DMAs instead of stalling the first real Ln.
            warm = singles.tile([P, 1], FP16)
            nc.scalar.activation(out=warm[:], in_=ew_t[:, 0, 0:1],
                                 func=mybir.ActivationFunctionType.Ln,
                                 bias=0.0, scale=1.0)

            # the 2^SCALE_K range shift is divided back out via fsc (host
            # folds 2^(-K/p) into the fsc scalars), so Exp needs no bias.
            # Ln/Exp run per half-batch (2 chunks) -- big enough to amortize
            # the ACT fixed cost, small enough to keep the pipeline draining.
            KH = 4
            PD = 2   # img prefetch distance (batches)

            # img loads are issued PD batches ahead of their compute so the
            # single SWDGE queue never stalls them behind out-stores (the Q7
            # sequencer waits each DMA's semaphore IN ORDER, so an out-store
            # waiting on its blend would otherwise starve later img loads).
            imgb_tiles = {}

            def issue_img(bb):
                t = imgp.tile([P, K, C * W], BF16, name="imgb")
                for k in range(K):
                    nc.gpsimd.dma_start(out=t[:, k, :],
                                        in_=img_r[bb, :, k])  # f32->bf16
                imgb_tiles[bb] = t

            for bb in range(min(PD, BL)):
                issue_img(bb)

            m_tiles = {}

            def merge_stage(b):
                """matmuls + power-domain merges + Ln/Exp for batch b."""
                m16b = m16p.tile([P, K, W], F32, name="m16b")
                for k in range(K):
                    ps = pp.tile([P, G, W], F32, name="ps")
                    for g in range(G):
                        nc.tensor.matmul(
                            out=ps[:, g, :], lhsT=gyT_t[:, b, k, :],
                            rhs=gxg_t[:, b, g * W:(g + 1) * W],
                            start=True, stop=True)
                    # serial max chain, one PSUM operand per op (PSUM has a
                    # single DVE read port)
                    if G > 1:
                        t = m16p.tile([P, G - 1, W], F32, tag="mrg",
                                      name="mrg")
                    prev = ew_t[:, k, :]
                    for g in range(G):
                        dst = m16b[:, k, :] if g == G - 1 else t[:, g, :]
                        nc.vector.tensor_tensor(
                            out=dst, in0=ps[:, g, :], in1=prev, op=mx)
                        prev = dst
                # m = exp(ln(.)/p); fp16 keeps |ln| <= 44 precise
                lnv = lnp.tile([P, K * W], FP16, name="lnv")
                nc.scalar.activation(
                    out=lnv[:], in_=m16b[:].rearrange("p k w -> p (k w)"),
                    func=mybir.ActivationFunctionType.Ln,
                    bias=0.0, scale=1.0)
                m_t = mbp.tile([P, K, W], BF16, name="m_t")
                nc.scalar.activation(
                    out=m_t[:].rearrange("p k w -> p (k w)"), in_=lnv[:],
                    func=mybir.ActivationFunctionType.Exp,
                    bias=0.0, scale=1.0 / PNORM)
                m_tiles[b] = m_t

            def blend_stage(b):
                """f + blend + out-store for batch b (runs one batch behind
                merge_stage so DVE never waits on ACT)."""
                imgb = imgb_tiles.pop(b)
                m_t = m_tiles.pop(b)
                for k in range(K):
                    # f_c = fsc_c * m + 1   (tensor_scalar)
                    f_t = fp.tile([P, C, W], BF16, name="f_t")
                    for c in range(C):
                        nc.vector.tensor_scalar(
                            out=f_t[:, c, :], in0=m_t[:, k, :],
                            scalar1=fsc_t[:, b, c:c + 1], scalar2=1.0,
                            op0=mult, op1=add)
                    # out = img * f  (one wide bf16 tt, 2x packed)
                    outb = outp.tile([P, C * W], BF16, name="outb")
                    nc.vector.tensor_tensor(
                        out=outb[:], in0=imgb[:, k, :],
                        in1=f_t[:].rearrange("p c w -> p (c w)"), op=mult)
                    nc.gpsimd.dma_start(out=out_r[b, :, k],
                                        in_=outb[:])  # bf16->f32

            for b in range(BL):
                if b + PD < BL:
                    issue_img(b + PD)
                merge_stage(b)
                if b >= 1:
                    blend_stage(b - 1)
            blend_stage(BL - 1)

    nc.compile()
    return nc


_NC_CACHE = {}


def _get_nc(G):
    if G not in _NC_CACHE:
        _NC_CACHE[G] = _build_program(G)
    return _NC_CACHE[G]


def _edge_weight16():
    y = np.linspace(-1.0, 1.0, H)
    x = np.linspace(-1.0, 1.0, W)
    yc, xc = np.meshgrid(y, x, indexing="ij")
    dist = np.sqrt(xc ** 2 + yc ** 2)
    ew = np.exp(2.0 * (dist - 0.7))
    ew = (ew - ew.min()) / (ew.max() - ew.min() + 1e-6)
    ew16 = np.maximum(ew ** PNORM, EW_FLOOR) * 2.0 ** SCALE_K
    lay = ew16.reshape(K, P, W).transpose(1, 0, 2).astype(ml_dtypes.bfloat16)
    return ew, np.ascontiguousarray(lay)


_EW = None          # (ew float64 [H,W], ew16 device layout [P,K,W] f32)


def _spot_tables(u_xy, u_radius, u_intensity):
    """gy[b,s,h], gx[b,s,w] (float64, sint folded into gx) and their ^p."""
    u_xy = np.asarray(u_xy, np.float64)
    u_radius = np.asarray(u_radius, np.float64)
    u_intensity = np.asarray(u_intensity, np.float64)

    y = np.linspace(-1.0, 1.0, H)
    x = np.linspace(-1.0, 1.0, W)
    spot_xy = 2.0 * u_xy - 1.0
    sx = spot_xy[..., 0]
    sy = spot_xy[..., 1]
    radius = 0.05 + 0.15 * u_radius
    sint = 0.5 + 0.5 * u_intensity
    inv2r2 = 1.0 / (2.0 * radius ** 2)

    dy2 = (y[None, None, :] - sy[..., None]) ** 2 * inv2r2[..., None]
    dx2 = (x[None, None, :] - sx[..., None]) ** 2 * inv2r2[..., None]
    gy = np.exp(-dy2)
    gx = np.exp(-dx2) * sint[..., None]
    # ^p computed in the exponent (no underflow-then-power double rounding)
    gyp = np.exp(-PNORM * dy2)
    gxp = np.exp(np.clip(-PNORM * dx2 + PNORM * np.log(sint[..., None]),
                         -745.0, 0.0))
    return gy, gx, gyp, gxp


def _out_err(m_apx, m_true, burn_b):
    fb = 0.7 * burn_b
    return np.max(np.abs(fb * (m_apx - m_true))
                  / (1.0 - fb * np.minimum(m_true, 1.0)))


def _group_spots(gyp_b, gxp_b, gy_b, gx_b, ew, G, burn_b):
    """Partition the 8 spots into <=G groups so the grouped p-norm tracks the
    true max.  Greedy coloring on a coarse conflict graph, then exhaustive
    search (minimizing the output-error bound) if greedy needs more than G
    colors."""
    ss = slice(0, H, 4)
    spots = gy_b[:, ss, None] * gx_b[:, None, ss]        # [S, 128, 128]
    sp = gyp_b[:, ss, None] * gxp_b[:, None, ss]         # == spots^p

    for thr in (0.006, 0.010):
        conflict = np.zeros((S, S), bool)
        for i in range(S):
            for j in range(i + 1, S):
                dev = (sp[i] + sp[j]) ** (1.0 / PNORM) \
                    - np.maximum(spots[i], spots[j])
                conflict[i, j] = conflict[j, i] = dev.max() > thr
        order = np.argsort(-conflict.sum(1))
        colors = -np.ones(S, int)
        ok = True
        for s in order:
            used = {colors[t] for t in range(S)
                    if conflict[s, t] and colors[t] >= 0}
            c0 = 0
            while c0 in used:
                c0 += 1
            if c0 >= G:
                ok = False
                break
            colors[s] = c0
        if ok:
            return colors

    # exhaustive over G-colorings (canonical: spot 0 in group 0), vectorized
    s8 = slice(0, 128, 2)          # stride-8 grid overall
    spc = spots[:, s8, :][:, :, s8].reshape(S, -1)
    spp = sp[:, s8, :][:, :, s8].reshape(S, -1)
    tmax = spc.max(0)
    best_dev, best_colors = np.inf, None
    n_assign = G ** (S - 1)
    for code in range(n_assign):
        colors = [0]
        c = code
        for _ in range(S - 1):
            colors.append(c % G)
            c //= G
        colors = np.array(colors)
        planes = np.stack([
            spp[colors == g].sum(0) if (colors == g).any()
            else np.zeros_like(tmax)
            for g in range(G)])
        dev = _out_err(planes.max(0) ** (1.0 / PNORM), tmax, burn_b)
        if dev < best_dev:
            best_dev, best_colors = dev, colors
    return best_colors


def _validate(colors, gyp_b, gxp_b, gy_b, gx_b, ew, G, burn_b):
    """stride-2 full check: grouped p-norm + ew vs true max + ew, scored as
    a bound on the output rel-err contribution."""
    ss = slice(0, H, 2)
    ew_s = ew[ss, :][:, ss]
    m_true = (gy_b[:, ss, None] * gx_b[:, None, ss]).max(0)
    m_true = np.maximum(m_true, ew_s)
    planes = np.stack([
        np.einsum('sh,sw->hw', gyp_b[colors == g][:, ss],
                  gxp_b[colors == g][:, ss])
        if (colors == g).any() else np.zeros_like(ew_s)
        for g in range(G)])
    m16 = np.maximum(planes.max(0), np.maximum(ew_s ** PNORM, EW_FLOOR))
    m_apx = m16 ** (1.0 / PNORM)
    return _out_err(m_apx, m_true, burn_b) <= OUT_BUDGET


def _host_tables(u_xy, u_radius, u_intensity, u_burn, ew):
    gy, gx, gyp, gxp = _spot_tables(u_xy, u_radius, u_intensity)
    u_burn = np.asarray(u_burn, np.float64)
    burn = BURN_MIN + (BURN_MAX - BURN_MIN) * u_burn
    fsc = -(burn[:, None] * (1.0 - DARK)[None, :])       # [B,C]

    G = 2
    while True:
        all_colors = np.zeros((B, S), int)
        ok = True
        for b in range(B):
            colors = _group_spots(gyp[b], gxp[b], gy[b], gx[b], ew, G,
                                  burn[b])
            if colors is None or not _validate(
                    colors, gyp[b], gxp[b], gy[b], gx[b], ew, G, burn[b]):
                ok = False
                break
            all_colors[b] = colors
        if ok:
            break
        G += 1                      # correctness escape hatch; recompiles
        if G > S:
            raise RuntimeError("spot grouping failed")

    # device layouts
    gyT_lay = np.ascontiguousarray(
        gyp.reshape(B, S, K, P).transpose(1, 0, 2, 3)
    ).astype(ml_dtypes.bfloat16)                          # [S,B,K,P]
    gxg_lay = np.zeros((S, B, G * W), dtype=ml_dtypes.bfloat16)
    gxp_dev = gxp * 2.0 ** SCALE_K      # range shift for the device Ln
    for b in range(B):
        for s in range(S):
            g = all_colors[b, s]
            gxg_lay[s, b, g * W:(g + 1) * W] = gxp_dev[b, s].astype(
                ml_dtypes.bfloat16)
    # fold the 2^(-SCALE_K/p) un-shift of the device mask into fsc
    fsc = fsc * 2.0 ** (-SCALE_K / PNORM)
    fsc_lay = np.broadcast_to(fsc.astype(np.float32), (P, B, C))
    return G, gyT_lay, gxg_lay, np.ascontiguousarray(fsc_lay)


def kernel(img, u_xy, u_radius, u_intensity, u_burn, _run_kwargs=None):
    global _EW
    img = np.ascontiguousarray(np.asarray(img, np.float32))
    if _EW is None:
        _EW = _edge_weight16()
    ew, ew16_lay = _EW

    G, gyT_lay, gxg_lay, fsc_lay = _host_tables(
        u_xy, u_radius, u_intensity, u_burn, ew)

    nc = _get_nc(G)
    core_ids = list(range(NCORES))
    in_maps = []
    for i in core_ids:
        lo, hi = i * BL, (i + 1) * BL
        in_maps.append({
            "img": img[lo:hi],
            "gyT": np.ascontiguousarray(gyT_lay[:, lo:hi]),
            "gxg": np.ascontiguousarray(gxg_lay[:, lo:hi]),
            "ew16": ew16_lay,
            "fsc": np.ascontiguousarray(fsc_lay[:, lo:hi]),
        })
    res = run_bass_kernel_spmd(nc, in_maps, core_ids, **(_run_kwargs or {}))
    out = np.concatenate([res.results[i]["out"] for i in core_ids], axis=0)
    if _run_kwargs:
        kernel._last_results = res
    return out


# revision 38
# speedup vs baseline: 1.1807x; 1.0117x over previous
"""Trainium2 Bass kernel for nn_PizzaBurningEffect.

Reference (per batch b):
    ew[h,w]   : fixed edge-weight grid (input-independent)
    spots     = max_s  gy_s[h] * gx_s[w]          (separable Gaussians)
    m         = max(ew, spots)
    out[c]    = img[c] * (1 + fsc_{b,c} * m),     fsc = -burn_b*(1-dark_c)
(The reference clips are no-ops: every operand is in [0,1) and bm <= 0.8.)

Device algorithm (p-norm max):
    max_s z_s  ==  (sum_s z_s^p)^(1/p)   when no two z_s tie at a large value.
    The host partitions the 8 spots into G groups (G=2 for the reference
    inputs; escalates per input until a per-pixel output-error bound is
    met), then the TensorEngine computes the per-group sums
    plane_g = sum_{s in g} gy_s^p * gx_s^p  as K=8 matmuls of the ^p 1-D
    tables (p=16).  DVE merges the G planes and ew^p with maxes *in the
    power domain* (max is monotone under x^(1/p)), ACT computes
    m = exp(ln(.)/p) once per batch, DVE computes f_c = 1 + fsc_c*m and
    the blend out = img * f in bf16.

    img/out are cast fp32<->bf16 during the DMA itself (SWDGE), so HBM
    traffic is the fp32 tensors but on-chip everything is 16-bit.  The
    kernel is HBM-bound: 25.8 MB/core at ~358 GB/s ~= 72 us is the floor;
    a 2-stage software pipeline (blend lags merge by one batch) plus
    2-batch img prefetch keep the single SWDGE queue from head-of-line
    blocking.

Error budget (measured on the reference inputs end to end): bf16 img/out
quantization ~8e-3, p-norm grouping deviation <= 1.1e-2 bound per pixel;
measured total 1.53e-2 vs the 2e-2 gate.  fp16 is NOT usable for img/out:
fp16 subnormal spacing (6e-8) vs the metric's 1e-6 denominator floor
gives 2e-2 error on its own.

Sharding: pure data parallel, 4 batches per core on 8 cores.
"""

import numpy as np
import ml_dtypes

import concourse.bacc as bacc
import concourse.bass as bass
from concourse import mybir
from concourse.tile import TileContext
from concourse.bass_utils import run_bass_kernel_spmd

# Make the act-table pass resolve both Ln and Exp to the one set that
# contains them both ("natural_log_exp_and_others"); the default order
# resolves them to two different sets, and the per-chunk Ln/Exp
# alternation then reloads tables every chunk (~1.5us each).
_orig_get_act_tables = bacc.get_activation_tables


def _pinned_act_tables(arch):
    # The entry ORDER must stay aligned with act_info.json (the index is the
    # act_func_set_id walrus resolves), so instead of reordering, strip
    # Ln/Exp from every other set so only the combined set can be chosen.
    tabs = _orig_get_act_tables(arch)
    key = "natural_log_exp_and_others"
    if key not in tabs:
        return tabs
    ln_exp = {mybir.ActivationFunctionType.Ln,
              mybir.ActivationFunctionType.Exp}
    return {name: (fns if name == key else fns - ln_exp)
            for name, fns in tabs.items()}


bacc.get_activation_tables = _pinned_act_tables

B, C, H, W, S = 32, 3, 512, 512, 8
NCORES = 8
BL = B // NCORES          # batches per core
P = 128                   # partitions
K = H // P                # row chunks per image
F32 = mybir.dt.float32
BF16 = mybir.dt.bfloat16
FP16 = mybir.dt.float16

BURN_MIN, BURN_MAX = 0.2, 0.8
DARK = np.array([0.7, 0.4, 0.3], dtype=np.float64)

PNORM = 16                # the p in the p-norm max
# The ACT Ln spline mishandles inputs below ~2^-64 (clamps, giving phantom
# mask ~0.06), so the power-domain values are pre-scaled by 2^SCALE_K and
# floored: m16*2^K stays in [2^-59, 2^62].  The scale is divided back out
# via Exp's free bias.  The floor makes the mask saturate at
# (2^-118)^(1/16) = 0.006 from below, well inside the error budget.
SCALE_K = 59
EW_FLOOR = 2.0 ** (-118)
# Validation accepts a grouping when the per-pixel OUTPUT error bound
# 0.7*burn*|m_apx-m_true| / (1 - 0.7*burn*m_true) stays under OUT_BUDGET;
# together with the ~0.8e-2 bf16 img/out quantization floor that keeps the
# end-to-end rel err under the 2e-2 gate.  Inputs that need more get a
# bigger G (recompile).
OUT_BUDGET = 1.15e-2


def _build_program(G):
    nc = bacc.Bacc("TRN2", target_bir_lowering=False, debug=False,
                   num_devices=NCORES)

    img = nc.dram_tensor("img", [BL, C, H, W], F32, kind="ExternalInput")
    gyT = nc.dram_tensor("gyT", [S, BL, K, P], BF16, kind="ExternalInput")
    gxg = nc.dram_tensor("gxg", [S, BL, G * W], BF16, kind="ExternalInput")
    ew16 = nc.dram_tensor("ew16", [P, K, W], BF16, kind="ExternalInput")
    fsc = nc.dram_tensor("fsc", [P, BL, C], F32, kind="ExternalInput")
    out = nc.dram_tensor("out", [BL, C, H, W], F32, kind="ExternalOutput")

    img_r = img.rearrange("b c (k p) w -> b p k c w", p=P)
    out_r = out.rearrange("b c (k p) w -> b p k c w", p=P)

    mx = mybir.AluOpType.max
    mult = mybir.AluOpType.mult
    add = mybir.AluOpType.add

    with TileContext(nc) as tc:
        with (
            tc.tile_pool(name="singles", bufs=1) as singles,
            tc.tile_pool(name="imgp", bufs=4) as imgp,
            tc.tile_pool(name="outp", bufs=8) as outp,
            tc.tile_pool(name="m16p", bufs=3) as m16p,
            tc.tile_pool(name="lnp", bufs=3) as lnp,
            tc.tile_pool(name="mbp", bufs=3) as mbp,
            tc.tile_pool(name="fp", bufs=4) as fp,
            tc.tile_pool(name="pp", bufs=4, space="PSUM") as pp,
        ):
            gyT_t = singles.tile([S, BL, K, P], BF16)
            nc.sync.dma_start(out=gyT_t[:], in_=gyT[:])
            gxg_t = singles.tile([S, BL, G * W], BF16)
            nc.sync.dma_start(out=gxg_t[:], in_=gxg[:])
            ew_t = singles.tile([P, K, W], BF16)
            nc.sync.dma_start(out=ew_t[:, 0, :], in_=ew16[:, 0, :])
            fsc_t = singles.tile([P, BL, C], F32)
            nc.sync.dma_start(out=fsc_t[:], in_=fsc[:])
            for k in range(1, K):
                nc.sync.dma_start(out=ew_t[:, k, :], in_=ew16[:, k, :])
            # Warm the ACT table (ln/exp set) so its ~2.7us load overlaps
            # the initial DMAs instead of stalling the first real Ln.
            warm = singles.tile([P, 1], FP16)
            nc.scalar.activation(out=warm[:], in_=ew_t[:, 0, 0:1],
                                 func=mybir.ActivationFunctionType.Ln,
                                 bias=0.0, scale=1.0)

            # the 2^SCALE_K range shift is divided back out via fsc (host
            # folds 2^(-K/p) into the fsc scalars), so Exp needs no bias.
            # Ln/Exp run per half-batch (2 chunks) -- big enough to amortize
            # the ACT fixed cost, small enough to keep the pipeline draining.
            KH = 4
            PD = 2   # img prefetch distance (batches)

            # img loads are issued PD batches ahead of their compute so the
            # single SWDGE queue never stalls them behind out-stores (the Q7
            # sequencer waits each DMA's semaphore IN ORDER, so an out-store
            # waiting on its blend would otherwise starve later img loads).
            imgb_tiles = {}

            def issue_img(bb):
                t = imgp.tile([P, K, C * W], BF16, name="imgb")
                for k in range(K):
                    nc.gpsimd.dma_start(out=t[:, k, :],
                                        in_=img_r[bb, :, k])  # f32->bf16
                imgb_tiles[bb] = t

            for bb in range(min(PD, BL)):
                issue_img(bb)

            m_tiles = {}

            def merge_stage(b):
                """matmuls + power-domain merges + Ln/Exp for batch b."""
                m16b = m16p.tile([P, K, W], F32, name="m16b")
                for k in range(K):
                    ps = pp.tile([P, G, W], F32, name="ps")
                    for g in range(G):
                        nc.tensor.matmul(
                            out=ps[:, g, :], lhsT=gyT_t[:, b, k, :],
                            rhs=gxg_t[:, b, g * W:(g + 1) * W],
                            start=True, stop=True)
                    # serial max chain, one PSUM operand per op (PSUM has a
                    # single DVE read port)
                    if G > 1:
                        t = m16p.tile([P, G - 1, W], F32, tag="mrg",
                                      name="mrg")
                    prev = ew_t[:, k, :]
                    for g in range(G):
                        dst = m16b[:, k, :] if g == G - 1 else t[:, g, :]
                        nc.vector.tensor_tensor(
                            out=dst, in0=ps[:, g, :], in1=prev, op=mx)
                        prev = dst
                # m = exp(ln(.)/p); fp16 keeps |ln| <= 44 precise
                lnv = lnp.tile([P, K * W], FP16, name="lnv")
                nc.scalar.activation(
                    out=lnv[:], in_=m16b[:].rearrange("p k w -> p (k w)"),
                    func=mybir.ActivationFunctionType.Ln,
                    bias=0.0, scale=1.0)
                m_t = mbp.tile([P, K, W], BF16, name="m_t")
                nc.scalar.activation(
                    out=m_t[:].rearrange("p k w -> p (k w)"), in_=lnv[:],
                    func=mybir.ActivationFunctionType.Exp,
                    bias=0.0, scale=1.0 / PNORM)
                m_tiles[b] = m_t

            def blend_stage(b):
                """f + blend + out-store for batch b (runs one batch behind
                merge_stage so DVE never waits on ACT)."""
                imgb = imgb_tiles.pop(b)
                m_t = m_tiles.pop(b)
                for k in range(K):
                    # f_c = fsc_c * m + 1   (tensor_scalar)
                    f_t = fp.tile([P, C, W], BF16, name="f_t")
                    for c in range(C):
                        nc.vector.tensor_scalar(
                            out=f_t[:, c, :], in0=m_t[:, k, :],
                            scalar1=fsc_t[:, b, c:c + 1], scalar2=1.0,
                            op0=mult, op1=add)
                    # out = img * f  (one wide bf16 tt, 2x packed)
                    outb = outp.tile([P, C * W], BF16, name="outb")
                    nc.vector.tensor_tensor(
                        out=outb[:], in0=imgb[:, k, :],
                        in1=f_t[:].rearrange("p c w -> p (c w)"), op=mult)
                    nc.gpsimd.dma_start(out=out_r[b, :, k],
                                        in_=outb[:])  # bf16->f32

            for b in range(BL):
                if b + PD < BL:
                    issue_img(b + PD)
                merge_stage(b)
                if b >= 1:
                    blend_stage(b - 1)
            blend_stage(BL - 1)

    nc.compile()
    return nc


_NC_CACHE = {}


def _get_nc(G):
    if G not in _NC_CACHE:
        _NC_CACHE[G] = _build_program(G)
    return _NC_CACHE[G]


def _edge_weight16():
    y = np.linspace(-1.0, 1.0, H)
    x = np.linspace(-1.0, 1.0, W)
    yc, xc = np.meshgrid(y, x, indexing="ij")
    dist = np.sqrt(xc ** 2 + yc ** 2)
    ew = np.exp(2.0 * (dist - 0.7))
    ew = (ew - ew.min()) / (ew.max() - ew.min() + 1e-6)
    ew16 = np.maximum(ew ** PNORM, EW_FLOOR) * 2.0 ** SCALE_K
    lay = ew16.reshape(K, P, W).transpose(1, 0, 2).astype(ml_dtypes.bfloat16)
    return ew, np.ascontiguousarray(lay)


_EW = None          # (ew float64 [H,W], ew16 device layout [P,K,W] f32)


def _spot_tables(u_xy, u_radius, u_intensity):
    """gy[b,s,h], gx[b,s,w] (float64, sint folded into gx) and their ^p."""
    u_xy = np.asarray(u_xy, np.float64)
    u_radius = np.asarray(u_radius, np.float64)
    u_intensity = np.asarray(u_intensity, np.float64)

    y = np.linspace(-1.0, 1.0, H)
    x = np.linspace(-1.0, 1.0, W)
    spot_xy = 2.0 * u_xy - 1.0
    sx = spot_xy[..., 0]
    sy = spot_xy[..., 1]
    radius = 0.05 + 0.15 * u_radius
    sint = 0.5 + 0.5 * u_intensity
    inv2r2 = 1.0 / (2.0 * radius ** 2)

    dy2 = (y[None, None, :] - sy[..., None]) ** 2 * inv2r2[..., None]
    dx2 = (x[None, None, :] - sx[..., None]) ** 2 * inv2r2[..., None]
    gy = np.exp(-dy2)
    gx = np.exp(-dx2) * sint[..., None]
    # ^p computed in the exponent (no underflow-then-power double rounding)
    gyp = np.exp(-PNORM * dy2)
    gxp = np.exp(np.clip(-PNORM * dx2 + PNORM * np.log(sint[..., None]),
                         -745.0, 0.0))
    return gy, gx, gyp, gxp


def _out_err(m_apx, m_true, burn_b):
    fb = 0.7 * burn_b
    return np.max(np.abs(fb * (m_apx - m_true))
                  / (1.0 - fb * np.minimum(m_true, 1.0)))


def _group_spots(gyp_b, gxp_b, gy_b, gx_b, ew, G, burn_b):
    """Partition the 8 spots into <=G groups so the grouped p-norm tracks the
    true max.  Greedy coloring on a coarse conflict graph, then exhaustive
    search (minimizing the output-error bound) if greedy needs more than G
    colors."""
    ss = slice(0, H, 4)
    spots = gy_b[:, ss, None] * gx_b[:, None, ss]        # [S, 128, 128]
    sp = gyp_b[:, ss, None] * gxp_b[:, None, ss]         # == spots^p

    for thr in (0.006, 0.010):
        conflict = np.zeros((S, S), bool)
        for i in range(S):
            for j in range(i + 1, S):
                dev = (sp[i] + sp[j]) ** (1.0 / PNORM) \
                    - np.maximum(spots[i], spots[j])
                conflict[i, j] = conflict[j, i] = dev.max() > thr
        order = np.argsort(-conflict.sum(1))
        colors = -np.ones(S, int)
        ok = True
        for s in order:
            used = {colors[t] for t in range(S)
                    if conflict[s, t] and colors[t] >= 0}
            c0 = 0
            while c0 in used:
                c0 += 1
            if c0 >= G:
                ok = False
                break
            colors[s] = c0
        if ok:
            return colors

    # exhaustive over G-colorings (canonical: spot 0 in group 0), vectorized
    s8 = slice(0, 128, 2)          # stride-8 grid overall
    spc = spots[:, s8, :][:, :, s8].reshape(S, -1)
    spp = sp[:, s8, :][:, :, s8].reshape(S, -1)
    tmax = spc.max(0)
    best_dev, best_colors = np.inf, None
    n_assign = G ** (S - 1)
    for code in range(n_assign):
        colors = [0]
        c = code
        for _ in range(S - 1):
            colors.append(c % G)
            c //= G
        colors = np.array(colors)
        planes = np.stack([
            spp[colors == g].sum(0) if (colors == g).any()
            else np.zeros_like(tmax)
            for g in range(G)])
        dev = _out_err(planes.max(0) ** (1.0 / PNORM), tmax, burn_b)
        if dev < best_dev:
            best_dev, best_colors = dev, colors
    return best_colors


def _validate(colors, gyp_b, gxp_b, gy_b, gx_b, ew, G, burn_b):
    """stride-2 full check: grouped p-norm + ew vs true max + ew, scored as
    a bound on the output rel-err contribution."""
    ss = slice(0, H, 2)
    ew_s = ew[ss, :][:, ss]
    m_true = (gy_b[:, ss, None] * gx_b[:, None, ss]).max(0)
    m_true = np.maximum(m_true, ew_s)
    planes = np.stack([
        np.einsum('sh,sw->hw', gyp_b[colors == g][:, ss],
                  gxp_b[colors == g][:, ss])
        if (colors == g).any() else np.zeros_like(ew_s)
        for g in range(G)])
    m16 = np.maximum(planes.max(0), np.maximum(ew_s ** PNORM, EW_FLOOR))
    m_apx = m16 ** (1.0 / PNORM)
    return _out_err(m_apx, m_true, burn_b) <= OUT_BUDGET


def _host_tables(u_xy, u_radius, u_intensity, u_burn, ew):
    gy, gx, gyp, gxp = _spot_tables(u_xy, u_radius, u_intensity)
    u_burn = np.asarray(u_burn, np.float64)
    burn = BURN_MIN + (BURN_MAX - BURN_MIN) * u_burn
    fsc = -(burn[:, None] * (1.0 - DARK)[None, :])       # [B,C]

    G = 2
    while True:
        all_colors = np.zeros((B, S), int)
        ok = True
        for b in range(B):
            colors = _group_spots(gyp[b], gxp[b], gy[b], gx[b], ew, G,
                                  burn[b])
            if colors is None or not _validate(
                    colors, gyp[b], gxp[b], gy[b], gx[b], ew, G, burn[b]):
                ok = False
                break
            all_colors[b] = colors
        if ok:
            break
        G += 1                      # correctness escape hatch; recompiles
        if G > S:
            raise RuntimeError("spot grouping failed")

    # device layouts
    gyT_lay = np.ascontiguousarray(
        gyp.reshape(B, S, K, P).transpose(1, 0, 2, 3)
    ).astype(ml_dtypes.bfloat16)                          # [S,B,K,P]
    gxg_lay = np.zeros((S, B, G * W), dtype=ml_dtypes.bfloat16)
    gxp_dev = gxp * 2.0 ** SCALE_K      # range shift for the device Ln
    for b in range(B):
        for s in range(S):
            g = all_colors[b, s]
            gxg_lay[s, b, g * W:(g + 1) * W] = gxp_dev[b, s].astype(
                ml_dtypes.bfloat16)
    # fold the 2^(-SCALE_K/p) un-shift of the device mask into fsc
    fsc = fsc * 2.0 ** (-SCALE_K / PNORM)
    fsc_lay = np.broadcast_to(fsc.astype(np.float32), (P, B, C))
    return G, gyT_lay, gxg_lay, np.ascontiguousarray(fsc_lay)


def kernel(img, u_xy, u_radius, u_intensity, u_burn, _run_kwargs=None):
    global _EW
    img = np.ascontiguousarray(np.asarray(img, np.float32))
    if _EW is None:
        _EW = _edge_weight16()
    ew, ew16_lay = _EW

    G, gyT_lay, gxg_lay, fsc_lay = _host_tables(
        u_xy, u_radius, u_intensity, u_burn, ew)

    nc = _get_nc(G)
    core_ids = list(range(NCORES))
    in_maps = []
    for i in core_ids:
        lo, hi = i * BL, (i + 1) * BL
        in_maps.append({
            "img": img[lo:hi],
            "gyT": np.ascontiguousarray(gyT_lay[:, lo:hi]),
            "gxg": np.ascontiguousarray(gxg_lay[:, lo:hi]),
            "ew16": ew16_lay,
            "fsc": np.ascontiguousarray(fsc_lay[:, lo:hi]),
        })
    res = run_bass_kernel_spmd(nc, in_maps, core_ids, **(_run_kwargs or {}))
    out = np.concatenate([res.results[i]["out"] for i in core_ids], axis=0)
    if _run_kwargs:
        kernel._last_results = res
    return out
